# revision 29
# baseline (speedup 1.0000x reference)
"""Bass/Tile kernel for nn_Attn_40424232189956 on 8 trn2 NeuronCores.

GQA attention block: q/k/v proj + rmsnorm + rope + causal attention + out proj.
B=2, T=2048, D=2048, NH=16, NKV=4, HD=128.

Sharding: 4 q-heads x 1 batch per core (core c: batch c//4, q heads
4*(c%4)..4*(c%4)+3, kv head c%4). Each (batch, kv head) pair is computed by
exactly one core -> no duplicated kv projection work. Each core emits a full
[T, D] partial of the output projection for its batch; host sums the 4
partials per batch.

Per-core kernel layout:
- Projections feat-major: psum [feat 128, tok 512], lhsT = W^T k-tiles,
  rhs = x^T k-tiles (x transposed + cast to bf16 on host). One batched DMA
  per 512-token chunk loads all 16 k-tiles.
- RMSNorm via ones-matmul partition reduction (value 1/(128*s_h^2) folds the
  qg gain and softmax 1/sqrt(HD) into the norm factor), sqrt bias eps/s_h^2.
- Rope in hd-major reading q halves straight from PSUM.
- Attention with TRANSPOSED scores sT[kt, qt]: softmax denominator via
  ones-column matmul (partition reduction on PE), p used directly as rhs of
  the pv matmul. exp() without max-subtraction (scores bounded by sqrt(HD)
  after rmsnorm). Heads processed in 2 passes of 2 (psum budget); the j-loop
  is software-pipelined: scores for step j issue before the sms/pv matmuls
  of step j-1, so the PE never waits on the exp() round trip.
- Causal masking: additive -30000 masks for the 4 diagonal block phases.
- Output written bf16 [T, D]; host sums partials in f32.
"""

import numpy as np

B, T, D = 2, 2048, 2048
NH, NKV = 16, 4
HD = 128
NCORES = 8
HPC = 4               # q heads per core
NKT = D // 128        # 16 contraction tiles for projections
CHUNK = 512
NCH = T // CHUNK      # 4 chunks
EPS = float(np.finfo(np.float32).eps)
MASK_NEG = -30000.0


def _rope_tables():
    # Matches reference.rotary_tables for T=2048 > tsl=1024 (NTK branch).
    hd = np.float32(HD)
    ar = (np.arange(0, HD, 2, dtype=np.float32) / hd).astype(np.float32)
    expo = np.power(np.float32(HD / (HD - 2.0)), ar, dtype=np.float32)
    inv = (np.float32(1.0)
           / (np.float32(10000.0)
              * np.power(np.float32(T / 1024.0), expo, dtype=np.float32)))
    f = np.outer(np.arange(T, dtype=np.float32), inv.astype(np.float32))
    return (np.cos(f).astype(np.float32).T.copy(),
            np.sin(f).astype(np.float32).T.copy())  # [64, T] hd-major


def _build_program():
    import concourse.bass as bass
    import concourse.mybir as mybir
    import concourse.tile as tile
    from concourse import bacc
    from concourse.masks import make_identity

    f32 = mybir.dt.float32
    f32r = mybir.dt.float32r
    bf16 = mybir.dt.bfloat16
    nc = bacc.Bacc("TRN2", target_bir_lowering=False)

    # lhsT (stationary) tensors are float32r: 4-byte weights self-load inside
    # the matmul, so tile_legalize emits no separate Ldweights instruction
    # (saves ~100ns of PE sequencer time per matmul). rhs (moving) tensors
    # are bf16: the PE row rate is keyed on the moving dtype.
    xT = nc.dram_tensor("xT", [D, T], bf16, kind="ExternalInput")
    qwT = nc.dram_tensor("qwT", [D, HPC * HD], f32r, kind="ExternalInput")
    kwT = nc.dram_tensor("kwT", [D, HD], f32r, kind="ExternalInput")
    vwT = nc.dram_tensor("vwT", [D, HD], f32r, kind="ExternalInput")
    owT = nc.dram_tensor("owT", [HPC * HD, D], bf16, kind="ExternalInput")
    csd = nc.dram_tensor("csd", [128, T], bf16, kind="ExternalInput")
    csd2 = nc.dram_tensor("csd2", [128, T], bf16, kind="ExternalInput")
    maskd = nc.dram_tensor("maskd", [128, 4, CHUNK], bf16, kind="ExternalInput")
    identd = nc.dram_tensor("identd", [128, 128], bf16, kind="ExternalInput")
    normod = nc.dram_tensor("normod", [128, HPC + 1, 128], f32r,
                            kind="ExternalInput")
    normbd = nc.dram_tensor("normbd", [128, HPC + 1], f32, kind="ExternalInput")
    onesd = nc.dram_tensor("onesd", [128, 1], f32r, kind="ExternalInput")
    outd = nc.dram_tensor("o", [T, D], bf16, kind="ExternalOutput")

    with tile.TileContext(nc) as tc:
        with (
            tc.tile_pool(name="wpool", bufs=1) as wpool,
            tc.tile_pool(name="xpool", bufs=4) as xpool,
            tc.tile_pool(name="big", bufs=1) as big,
            tc.tile_pool(name="ybp", bufs=2) as ybp,
            tc.tile_pool(name="ntmp", bufs=3) as ntmp,
            tc.tile_pool(name="ntmp1", bufs=3) as ntmp1,
            tc.tile_pool(name="atmp", bufs=2) as atmp,
            tc.tile_pool(name="ppool", bufs=4) as ppool,
            tc.tile_pool(name="opool", bufs=2) as opool,
            tc.tile_pool(name="ps", bufs=6, space="PSUM") as ps,
            tc.tile_pool(name="psv", bufs=2, space="PSUM") as psv,
        ):
            # ---- resident weights / tables ----
            # x and qw quarter-loads are interleaved on the SP queue in the
            # order the projection consumes them (ko-major); bulky tables
            # that are needed later (rope cos/sin, ow, mask) go out on the
            # DVE hardware-DGE queue so they never starve the x stream.
            NQ = NKT // 4
            xr = xT.rearrange("(ko p) m -> p ko m", p=128)
            qwr = qwT.rearrange("(ko p) m -> p ko m", p=128)
            kwr = kwT.rearrange("(ko p) m -> p ko m", p=128)
            vwr = vwT.rearrange("(ko p) m -> p ko m", p=128)
            # one tile per DMA piece: dependency tracking is whole-tile, so a
            # single big weight tile would stall the first matmul on the LAST
            # piece's arrival
            # x lives in dram as bf16 (half the transfer bytes) and is cast
            # to f32r during the gpsimd DMA, keeping the SP queue free for
            # output writes
            xc0 = xpool.tile([128, NQ, CHUNK], f32r, tag="xc",
                             name="xc_0_0")
            nc.gpsimd.dma_start(xc0[:], xr[:, 0:NQ, 0:CHUNK])
            qw_p = [wpool.tile([128, 2, HPC * HD], f32r, name=f"qw_p{i}")
                    for i in range(8)]
            kw_p = [wpool.tile([128, 8, HD], f32r, name=f"kw_p{i}")
                    for i in range(2)]
            vw_p = [wpool.tile([128, 8, HD], f32r, name=f"vw_p{i}")
                    for i in range(2)]
            nc.scalar.dma_start(qw_p[0][:], qwr[:, 0:2, :])
            nc.scalar.dma_start(kw_p[0][:], kwr[:, 0:8, :])
            nc.scalar.dma_start(vw_p[0][:], vwr[:, 0:8, :])
            nc.scalar.dma_start(qw_p[1][:], qwr[:, 2:4, :])
            nc.scalar.dma_start(kw_p[1][:], kwr[:, 8:16, :])
            nc.scalar.dma_start(vw_p[1][:], vwr[:, 8:16, :])
            for qq in range(2, 8):
                nc.scalar.dma_start(qw_p[qq][:],
                                    qwr[:, 2 * qq:2 * qq + 2, :])

            def qw_at(ko, h):
                return qw_p[ko // 2][:, ko % 2, h * 128:(h + 1) * 128]

            def kw_at(ko):
                return kw_p[ko // 8][:, ko % 8, :]

            def vw_at(ko):
                return vw_p[ko // 8][:, ko % 8, :]
            xc0q = [xc0]
            for qq in range(1, 4):
                xc0q.append(xpool.tile([128, NQ, CHUNK], f32r, tag="xc",
                                       name=f"xc_0_{qq}"))
                nc.gpsimd.dma_start(xc0q[qq][:],
                                    xr[:, NQ * qq:NQ * (qq + 1), 0:CHUNK])
            normo_s = wpool.tile([128, HPC + 1, 128], f32r)
            nc.gpsimd.dma_start(normo_s[:], normod[:])
            normb_s = wpool.tile([128, HPC + 1], f32)
            nc.gpsimd.dma_start(normb_s[:], normbd[:])
            ones_col = wpool.tile([128, 1], f32r)
            nc.gpsimd.dma_start(ones_col[:], onesd[:])
            # bulk tables on the Pool software-DGE queue (idle at startup)
            cs_s = wpool.tile([128, T], bf16)  # rows 0:64 cos, 64:128 sin
            nc.gpsimd.dma_start(cs_s[:], csd[:])
            cs2_s = wpool.tile([128, T], bf16)  # rows 0:64 sin, 64:128 cos
            nc.gpsimd.dma_start(cs2_s[:], csd2[:])
            mask_s = wpool.tile([128, 4, CHUNK], bf16)
            nc.gpsimd.dma_start(mask_s[:], maskd[:])
            identneg = wpool.tile([128, 128], bf16)
            nc.gpsimd.dma_start(identneg[:], identd[:])
            ow_s = wpool.tile([128, HPC, D], bf16)
            nc.gpsimd.dma_start(ow_s[:], owT.rearrange("(h p) n -> p h n", p=128))
            ident = wpool.tile([128, 128], f32)
            make_identity(nc, ident[:])

            # Separate tiles per 512-token chunk: the Tile framework tracks
            # dependencies at whole-tile granularity, so a single [128, T]
            # tensor would falsely serialize attention group c on the rope
            # writes of later chunks.
            qTt = [big.tile([128, HPC, CHUNK], bf16, tag=f"qT{c}",
                            name=f"qT{c}") for c in range(NCH)]
            kTt = [big.tile([128, CHUNK], bf16, tag=f"kT{c}",
                            name=f"kT{c}") for c in range(NCH)]
            vtokt = [big.tile([128, CHUNK], f32r, tag=f"vtok{c}",
                              name=f"vtok{c}") for c in range(NCH)]

            sq_ = mybir.ActivationFunctionType.Square
            sqrt_ = mybir.ActivationFunctionType.Sqrt
            exp_ = mybir.ActivationFunctionType.Exp

            def norm_front(pt, ni, pos0):
                """pt: psum [128 feat, 512 tok]. DVE-only front half: square,
                copy to sbuf (frees the psum bank), partition-reduce on PE,
                stats to sbuf. Keeps the ACT engine free for sqrt/exp only,
                so activation-table reloads happen just twice per chunk."""
                qsb = ntmp.tile([128, CHUNK], f32, tag="qsb")
                nc.vector.tensor_copy(out=qsb[:], in_=pt[:])
                sq = ntmp.tile([128, CHUNK], f32r, tag="sq")
                nc.vector.tensor_mul(sq[:], qsb[:], pt[:])
                nb = psv.tile([128, CHUNK], f32, tag="aux", name=f"nb_{ni}_{pos0}")
                nc.tensor.matmul(nb[:], normo_s[:, ni, :], sq[:],
                                 start=True, stop=True)
                nbs = ntmp.tile([64, CHUNK], f32, tag="nbs")
                nc.vector.tensor_copy(out=nbs[:], in_=nb[0:64, :])
                return qsb, nbs

            def norm_back(front, ni, dst, pos0):
                qsb, nbs = front
                rs = ntmp1.tile([64, CHUNK], f32, tag="rs")
                nc.scalar.activation(out=rs[:], in_=nbs[:], func=sqrt_,
                                     bias=normb_s[0:64, ni:ni + 1], scale=1.0)
                rfac = rs
                nc.vector.reciprocal(rfac[:], rs[:])
                cs = cs_s[0:64, pos0:pos0 + CHUNK]       # cos @ base 0
                sn = cs_s[64:128, pos0:pos0 + CHUNK]     # sin @ base 64
                sn0 = cs2_s[0:64, pos0:pos0 + CHUNK]     # sin @ base 0
                cs64 = cs2_s[64:128, pos0:pos0 + CHUNK]  # cos @ base 64
                # multiplies on the GPSIMD engine (sbuf-only operands; each
                # operand pair shares a partition window)
                t1 = ntmp1.tile([64, CHUNK], f32, tag="ta")
                t2 = ntmp1.tile([64, CHUNK], f32, tag="tb")
                nc.gpsimd.tensor_mul(t1[:], qsb[0:64, :], cs)
                nc.gpsimd.tensor_mul(t2[:], qsb[64:128, :], sn)
                nc.vector.tensor_add(t1[:], t1[:], t2[:])
                nc.vector.tensor_mul(dst[0:64, :], t1[:], rfac[:])
                t3 = ntmp1.tile([64, CHUNK], f32, tag="tb")
                t4 = ntmp1.tile([64, CHUNK], f32, tag="ta")
                nc.gpsimd.tensor_mul(t3[:], qsb[0:64, :], sn0)
                nc.gpsimd.tensor_mul(t4[:], qsb[64:128, :], cs64)
                nc.vector.tensor_sub(t4[:], t4[:], t3[:])
                nc.vector.tensor_mul(dst[64:128, :], t4[:], rfac[:])

            def proj_chunk(ci):
                pos0 = ci * CHUNK
                if ci == 0:
                    xh = xc0q
                else:
                    xh = []
                    for qq in range(4):
                        xh.append(xpool.tile([128, NQ, CHUNK], f32r, tag="xc",
                                             name=f"xc_{ci}_{qq}"))
                        nc.gpsimd.dma_start(
                            xh[qq][:],
                            xr[:, NQ * qq:NQ * (qq + 1), pos0:pos0 + CHUNK])
                # two psum phases of 3 banks each: phase-A norms (K, q0, q1)
                # run on ACT/DVE/Pool while phase-B matmuls occupy the PE, so
                # the next attention group's k/q tiles are ready sooner and
                # the next proj chunk's psum frees early.
                pk = ps.tile([128, CHUNK], f32, tag="b512", name=f"pk_{ci}")
                pq = [ps.tile([128, CHUNK], f32, tag="b512",
                              name=f"pq{h}_{ci}") for h in range(2)]
                for ko in range(NKT):
                    rhs = xh[ko // NQ][:, ko % NQ, :]
                    st = (ko == 0)
                    sp = (ko == NKT - 1)
                    for h in range(2):
                        nc.tensor.matmul(pq[h][:], qw_at(ko, h),
                                         rhs, start=st, stop=sp)
                    nc.tensor.matmul(pk[:], kw_at(ko), rhs,
                                     start=st, stop=sp)
                fK = norm_front(pk, HPC, pos0)
                f0 = norm_front(pq[0], 0, pos0)
                f1 = norm_front(pq[1], 1, pos0)
                norm_back(fK, HPC, kTt[ci][:], pos0)
                norm_back(f0, 0, qTt[ci][:, 0, :], pos0)
                norm_back(f1, 1, qTt[ci][:, 1, :], pos0)
                pv = ps.tile([128, CHUNK], f32, tag="b512", name=f"pv_{ci}")
                pq2 = [ps.tile([128, CHUNK], f32, tag="b512",
                               name=f"pq{h}_{ci}") for h in (2, 3)]
                for ko in range(NKT):
                    rhs = xh[ko // NQ][:, ko % NQ, :]
                    st = (ko == 0)
                    sp = (ko == NKT - 1)
                    nc.tensor.matmul(pv[:], vw_at(ko), rhs,
                                     start=st, stop=sp)
                    for i, h in enumerate((2, 3)):
                        nc.tensor.matmul(pq2[i][:], qw_at(ko, h),
                                         rhs, start=st, stop=sp)
                # v: psum [hd, tok] -> sbuf (DVE), PE-transpose to token-major
                vtmp = atmp.tile([128, CHUNK], f32, tag="vtmp",
                                 name=f"vtmp_{ci}")
                nc.vector.tensor_copy(out=vtmp[:], in_=pv[:])
                for tb in range(4):
                    vps = psv.tile([128, CHUNK], f32, tag="aux",
                                   name=f"vps_{ci}_{tb}")[:, 0:128]
                    nc.tensor.transpose(
                        vps, vtmp[:, tb * 128:(tb + 1) * 128], ident[:])
                    dst0 = tb * 128
                    nc.vector.tensor_copy(out=vtokt[ci][:, dst0:dst0 + 128],
                                          in_=vps)
                f2 = norm_front(pq2[0], 2, pos0)
                f3 = norm_front(pq2[1], 3, pos0)
                norm_back(f2, 2, qTt[ci][:, 2, :], pos0)
                norm_back(f3, 3, qTt[ci][:, 3, :], pos0)

            def attn_pass(g, hh, ybg):
                """Attention for query group g, heads hh (pair). Writes
                normalized per-head outputs into ybg[:, h, :]."""
                q0 = g * CHUNK
                kg = 4 * (g + 1)
                yts, sms = {}, {}
                for h in hh:
                    yts[h] = ps.tile([128, CHUNK], f32, tag="b512",
                                     name=f"yt_{g}_{h}")
                    sms[h] = psv.tile([128, CHUNK], f32, tag="aux",
                                      name=f"sm_{g}_{h}")[0:1, :]
                pend = None  # (j, {h: pj}) awaiting sms/pv issue
                for j in range(kg):
                    cj, lj = divmod(j, 4)
                    k0 = lj * 128
                    st = {}
                    diag = j >= 4 * g
                    for h in hh:  # both scores share the kT lhsT
                        st[h] = ps.tile([128, CHUNK], f32, tag="b512",
                                        name=f"st_{g}_{h}_{j}")
                        nc.tensor.matmul(st[h][:], kTt[cj][:, k0:k0 + 128],
                                         qTt[g][:, h, :],
                                         start=True, stop=not diag)
                        if diag:
                            # causal mask on the PE: -30000*I @ mask01
                            # accumulates the additive mask into the score
                            # bank, keeping the DVE out of the exp chain
                            nc.tensor.matmul(st[h][:], identneg[:],
                                             mask_s[:, j - 4 * g, :],
                                             start=False, stop=True,
                                             skip_group_check=True)
                    if pend is not None:
                        pj_, j_ = pend
                        cp, lp = divmod(j_, 4)
                        for h in hh:
                            nc.tensor.matmul(sms[h], ones_col[:], pj_[h][:],
                                             start=(j_ == 0), stop=False,
                                             skip_group_check=True)
                        for h in hh:
                            nc.tensor.matmul(yts[h][:],
                                             vtokt[cp][:, lp * 128:lp * 128 + 128],
                                             pj_[h][:],
                                             start=(j_ == 0), stop=False,
                                             skip_group_check=True)
                    pjs = {}
                    for h in hh:
                        pj = ppool.tile([128, CHUNK], f32r, tag="pj",
                                        name=f"pj_{g}_{h}_{j}")
                        nc.scalar.activation(out=pj[:], in_=st[h][:],
                                             func=exp_)
                        pjs[h] = pj
                    pend = (pjs, j)
                pj_, j_ = pend
                cp, lp = divmod(j_, 4)
                for h in hh:
                    nc.tensor.matmul(sms[h], ones_col[:], pj_[h][:],
                                     start=(j_ == 0), stop=True,
                                     skip_group_check=True)
                for h in hh:
                    nc.tensor.matmul(yts[h][:],
                                     vtokt[cp][:, lp * 128:lp * 128 + 128],
                                     pj_[h][:], start=(j_ == 0), stop=True,
                                     skip_group_check=True)
                for h in hh:
                    rrow = ntmp1.tile([1, CHUNK], f32, tag="rs",
                                     name=f"rr_{g}_{h}")
                    nc.vector.reciprocal(rrow[:], sms[h])
                    rb = atmp.tile([128, CHUNK], f32, tag="rb",
                                   name=f"rb_{g}_{h}")
                    nc.gpsimd.partition_broadcast(rb[:], rrow[:])
                    nc.vector.tensor_mul(ybg[:, h, :], yts[h][:], rb[:])

            def oproj_group(g, ybg):
                q0 = g * CHUNK
                for tb in range(4):
                    row0 = q0 + tb * 128
                    ops = [ps.tile([128, CHUNK], f32, tag="b512",
                                   name=f"op_{g}_{tb}_{oc}")
                           for oc in range(4)]
                    for h in range(HPC):
                        lhsT = ybg[:, h, tb * 128:(tb + 1) * 128]
                        for oc in range(4):
                            nc.tensor.matmul(
                                ops[oc][:], lhsT,
                                ow_s[:, h, oc * 512:(oc + 1) * 512],
                                start=(h == 0), stop=(h == HPC - 1),
                                skip_group_check=True)
                    orow = opool.tile([128, D], bf16, tag="orow",
                                      name=f"or_{g}_{tb}")
                    for oc in range(4):
                        dst = orow[:, oc * 512:(oc + 1) * 512]
                        nc.vector.tensor_copy(out=dst, in_=ops[oc][:])
                    nc.sync.dma_start(outd[row0:row0 + 128, :], orow[:])

            # Schedule: proj of chunk c+2 and the output projection of group
            # c are emitted between attention passes so that (a) rope chains
            # resolve while the PE runs attention and (b) the softmax
            # epilogue (recip/broadcast/normalize) of a pass resolves while
            # the PE runs the next proj chunk, not while oproj waits on it.
            proj_chunk(0)
            proj_chunk(1)
            ybgs = [ybp.tile([128, HPC, CHUNK], bf16, tag="ybg",
                             name=f"ybg_{c}") for c in range(NCH)]
            attn_pass(0, (0, 1), ybgs[0])
            attn_pass(0, (2, 3), ybgs[0])
            proj_chunk(2)
            attn_pass(1, (0, 1), ybgs[1])
            attn_pass(1, (2, 3), ybgs[1])
            proj_chunk(3)
            oproj_group(0, ybgs[0])
            attn_pass(2, (0, 1), ybgs[2])
            attn_pass(2, (2, 3), ybgs[2])
            oproj_group(1, ybgs[1])
            attn_pass(3, (0, 1), ybgs[3])
            attn_pass(3, (2, 3), ybgs[3])
            oproj_group(2, ybgs[2])
            oproj_group(3, ybgs[3])

    nc.compile()
    return nc


_CACHED = {}
LAST_EXEC_NS = None


def _run(nc, in_maps, **kwargs):
    from concourse.bass_utils import run_bass_kernel_spmd
    return run_bass_kernel_spmd(nc, in_maps, core_ids=list(range(NCORES)),
                                **kwargs)


def _make_in_maps(x, qw, kw, vw, ow, qg):
    import ml_dtypes
    bf = ml_dtypes.bfloat16
    cosT, sinT = _rope_tables()
    cossin = np.concatenate([cosT, sinT], axis=0).astype(bf)
    sincos = np.concatenate([sinT, cosT], axis=0).astype(bf)

    ktl = np.arange(128, dtype=np.int64)[:, None]
    qtl = np.arange(CHUNK, dtype=np.int64)[None, :]
    mask = np.zeros((128, 4, CHUNK), np.float32)
    for r in range(4):
        mask[:, r, :] = np.where(qtl >= ktl + 128 * r, 0.0, 1.0)
    mask = mask.astype(bf)
    identneg = (MASK_NEG * np.eye(128)).astype(bf)

    xTb = [np.ascontiguousarray(x[b].T).astype(bf) for b in range(B)]

    in_maps = []
    for c in range(NCORES):
        bi, hg = divmod(c, HPC)
        h0 = HPC * hg
        qwT_c = np.ascontiguousarray(qw[h0 * HD:(h0 + HPC) * HD, :].T)
        kwT_c = np.ascontiguousarray(kw[hg * HD:(hg + 1) * HD, :].T)
        vwT_c = np.ascontiguousarray(vw[hg * HD:(hg + 1) * HD, :].T)
        owT_c = ow[:, h0 * HD:(h0 + HPC) * HD].T.astype(bf).copy()
        # norm constants: s_i folds qg gain and 1/sqrt(HD) attention scale
        s = np.array([qg[h0 + i] / np.sqrt(HD) for i in range(HPC)] + [1.0],
                     np.float32)
        normo = np.broadcast_to(
            (1.0 / (HD * s * s))[None, :, None], (128, HPC + 1, 128)
        ).astype(np.float32).copy()
        normb = np.broadcast_to(
            (EPS / (s * s))[None, :], (128, HPC + 1)).astype(np.float32).copy()
        in_maps.append({
            "xT": xTb[bi], "qwT": qwT_c, "kwT": kwT_c, "vwT": vwT_c,
            "owT": owT_c, "csd": cossin, "csd2": sincos, "maskd": mask,
            "normod": normo, "normbd": normb,
            "onesd": np.ones((128, 1), np.float32),
            "identd": identneg,
        })
    return in_maps


def kernel(x, qw, kw, vw, ow, qg):
    global LAST_EXEC_NS
    x = np.ascontiguousarray(x, dtype=np.float32)
    qw = np.asarray(qw, dtype=np.float32)
    kw = np.asarray(kw, dtype=np.float32)
    vw = np.asarray(vw, dtype=np.float32)
    ow = np.asarray(ow, dtype=np.float32)
    qg = np.asarray(qg, dtype=np.float32)

    if "nc" not in _CACHED:
        _CACHED["nc"] = _build_program()
    nc = _CACHED["nc"]

    in_maps = _make_in_maps(x, qw, kw, vw, ow, qg)
    res = _run(nc, in_maps)
    LAST_EXEC_NS = res.exec_time_ns
    out = np.zeros((B, T, D), np.float64)
    for c in range(NCORES):
        bi = c // HPC
        out[bi] += res.results[c]["o"].astype(np.float64)
    return out.astype(np.float32)
